# revision 3
# baseline (speedup 1.0000x reference)
"""Trainium2 Bass kernel: batch-parallel tanh-projected attention.

Reference (per batch element, 8 elements total):
    qh = tanh(q @ Wq + bq); kh = tanh(k @ Wk + bk); vh = tanh(v @ Wv + bv)
    out = softmax(qh @ kh^T, axis=-1) @ vh

Sharding: data-parallel over batch B=8 across the 8 NeuronCores; the small
256x32 projection weights are replicated.

Per-core algorithm (all in "transposed" layouts to avoid transposing the
2048x2048 attention matrix):
  - Cast q/k/v to bf16 during DMA (SWDGE), PE-transpose 128x128 tiles to get
    qT/kT/vT = [DIN, n] in SBUF.
  - Projections compute hT4 = [128, 2048] where partition 32*i + c holds
    channel c of qh^T/kh^T (replicated 4x along partitions via replicated
    weight columns) -> enables 4-way TensorE row-group packing for the K=32
    score matmuls.
  - S^T = kh @ qh^T computed key-tile by key-tile ([128 keys, 512 q] PSUM),
    exp on ScalarE without max-subtraction (scores bounded by |S| <= 32
    since tanh outputs are in (-1,1); measured |S| ~ 13).
  - O^T accumulated as [vh | 1]^T @ exp(S^T): the ones column yields the
    softmax denominator for free. 2-way column-group packing (M=33 -> 64-
    aligned column strips).
  - PE-transpose O^T chunks, divide by the denominator, DMA out.
"""

import numpy as np

B, N, M, DIN, DH = 8, 2048, 2048, 256, 32
P = 128
NT = N // P  # 16 row tiles
QC = 512  # q-chunk (matmul moving dim)
NQC = N // QC  # 4


def _build():
    import concourse.bass as bass
    import concourse.mybir as mybir
    import concourse.tile as tile
    from concourse import bacc
    from concourse.masks import make_identity

    fp32 = mybir.dt.float32
    bf16 = mybir.dt.bfloat16

    nc = bacc.Bacc("TRN2", target_bir_lowering=False, debug=False)

    q_d = nc.dram_tensor("q", [N, DIN], fp32, kind="ExternalInput")
    k_d = nc.dram_tensor("k", [M, DIN], fp32, kind="ExternalInput")
    v_d = nc.dram_tensor("v", [M, DIN], fp32, kind="ExternalInput")
    wq_d = nc.dram_tensor("Wq", [DIN, DH], fp32, kind="ExternalInput")
    wk_d = nc.dram_tensor("Wk", [DIN, DH], fp32, kind="ExternalInput")
    wv_d = nc.dram_tensor("Wv", [DIN, DH], fp32, kind="ExternalInput")
    bq_d = nc.dram_tensor("bq", [DH], fp32, kind="ExternalInput")
    bk_d = nc.dram_tensor("bk", [DH], fp32, kind="ExternalInput")
    bv_d = nc.dram_tensor("bv", [DH], fp32, kind="ExternalInput")
    out_d = nc.dram_tensor("out", [N, DH], fp32, kind="ExternalOutput")

    with tile.TileContext(nc) as tc:
        with (
            tc.tile_pool(name="const", bufs=1) as const,
            tc.tile_pool(name="stage", bufs=6) as stage,
            tc.tile_pool(name="sb", bufs=1) as sb,
            tc.tile_pool(name="expp", bufs=3) as expp,
            tc.tile_pool(name="osb", bufs=2) as osb,
            tc.tile_pool(name="pbig", bufs=2, space="PSUM") as pbig,
            tc.tile_pool(name="po", bufs=2, space="PSUM") as po,
            tc.tile_pool(name="pt2", bufs=2, space="PSUM") as pt2,
        ):
            # ---- constants ----
            id_bf = const.tile([P, P], bf16)
            make_identity(nc, id_bf[:])
            id_f32 = const.tile([P, P], fp32)
            make_identity(nc, id_f32[:])

            # weights: load f32 then cast+replicate 4x along output channels
            w4 = {}
            for name, wd in (("q", wq_d), ("k", wk_d), ("v", wv_d)):
                wf = const.tile([P, 2, DH], fp32, tag=f"wf_{name}")
                nc.sync.dma_start(wf[:], wd[:].rearrange("(o p) c -> p o c", p=P))
                w4t = const.tile([P, 2, 4 * DH], bf16, tag=f"w4_{name}")
                for j in range(4):
                    nc.vector.tensor_copy(w4t[:, :, j * DH : (j + 1) * DH], wf[:])
                w4[name] = w4t

            bias = {}
            for name, bd in (("q", bq_d), ("k", bk_d), ("v", bv_d)):
                bt = const.tile([P, 1], fp32, tag=f"b_{name}")
                for i in range(4):
                    nc.sync.dma_start(
                        bt[i * DH : (i + 1) * DH, :],
                        bd[:].rearrange("(c one) -> c one", one=1),
                    )
                bias[name] = bt

            # ---- input load (cast f32->bf16 in DMA) + PE transpose ----
            # xT layout: [P, 2, N] bf16, xT[p, o, n] = x[n, o*128 + p]
            xT = {}
            for name, xd in (("q", q_d), ("k", k_d), ("v", v_d)):
                xT[name] = sb.tile([P, 2, N], bf16, tag=f"xT_{name}", name=f"xT_{name}")
            for name, xd in (("q", q_d), ("k", k_d), ("v", v_d)):
                src = xd[:].rearrange("(t p) d -> p t d", p=P)
                for g in range(4):  # groups of 4 row-tiles
                    xbf = stage.tile([P, 4, DIN], bf16, tag="xbf")
                    nc.gpsimd.dma_start(xbf[:], src[:, 4 * g : 4 * g + 4, :])
                    for o in range(2):
                        ptp = pbig.tile([P, 4, P], bf16, tag="big")
                        for i in range(4):
                            nc.tensor.transpose(
                                ptp[:, i, :],
                                xbf[:, i, o * P : (o + 1) * P],
                                id_bf[:],
                            )
                        nc.vector.tensor_copy(
                            xT[name][:, o, 512 * g : 512 * (g + 1)], ptp[:]
                        )

            # ---- projections: hT4[name] = [P, N] bf16, 4x replicated ----
            hT4 = {}
            for name in ("q", "k", "v"):
                hT4[name] = sb.tile([P, N], bf16, tag=f"hT4_{name}", name=f"hT4_{name}")
            for name in ("q", "k", "v"):
                for ch in range(2):  # halves of N
                    ph = pbig.tile([P, 2, QC], fp32, tag="big")
                    for nh in range(2):
                        for o in range(2):
                            nc.tensor.matmul(
                                ph[:, nh, :],
                                w4[name][:, o, :],
                                xT[name][:, o, 1024 * ch + 512 * nh : 1024 * ch + 512 * (nh + 1)],
                                start=(o == 0),
                                stop=(o == 1),
                            )
                    nc.scalar.activation(
                        hT4[name][:, 1024 * ch : 1024 * (ch + 1)].rearrange(
                            "p (a b) -> p a b", a=2
                        ),
                        ph[:],
                        mybir.ActivationFunctionType.Tanh,
                        bias=bias[name][:],
                    )

            # ---- vh_aug: [P, NT, DH+1] bf16 (row-major vh tiles + ones col) ----
            vh_aug = sb.tile([P, NT, DH + 1], bf16)
            nc.gpsimd.memset(vh_aug[:, :, DH : DH + 1], 1.0)
            for g in range(4):
                pv = pbig.tile([P, 4, DH], bf16, tag="big")
                for i in range(4):
                    kt = 4 * g + i
                    nc.tensor.transpose(
                        pv[:, i, :],
                        hT4["v"][0:DH, P * kt : P * (kt + 1)],
                        id_bf[0:DH, 0:DH],
                    )
                nc.vector.tensor_copy(vh_aug[:, 4 * g : 4 * g + 4, 0:DH], pv[:])

            # ---- main attention loop ----
            out_sb = sb.tile([P, NT, DH], fp32)
            for c in range(NQC):
                qs = slice(QC * c, QC * (c + 1))
                po_t = po.tile([P, QC], fp32)
                for r in range(8):  # pairs of key tiles
                    pT = pbig.tile([P, 2, QC], fp32, tag="big")
                    for i in range(2):
                        kt = 2 * r + i
                        rg = kt % 4
                        nc.tensor.matmul(
                            pT[:, i, :],
                            hT4["k"][32 * rg : 32 * (rg + 1), P * kt : P * (kt + 1)],
                            hT4["q"][32 * rg : 32 * (rg + 1), qs],
                            start=True,
                            stop=True,
                            tile_position=(32 * rg, 0),
                        )
                    eT = expp.tile([P, 2, QC], bf16, tag="exp")
                    nc.scalar.activation(
                        eT[:], pT[:], mybir.ActivationFunctionType.Exp
                    )
                    for i in range(2):
                        kt = 2 * r + i
                        cg = kt % 2
                        nc.tensor.matmul(
                            po_t[64 * cg : 64 * cg + DH + 1, :],
                            vh_aug[:, kt, :],
                            eT[:, i, :],
                            start=(kt < 2),
                            stop=(kt >= 2 * 8 - 2),
                            tile_position=(0, 64 * cg),
                        )
                # combine col groups, transpose, normalize, stage output
                o_sb = osb.tile([DH + 1, QC], fp32, tag="o_sb")
                nc.vector.tensor_copy(o_sb[:], po_t[0 : DH + 1, :])
                nc.vector.tensor_add(
                    o_sb[:], o_sb[:], po_t[64 : 64 + DH + 1, :]
                )
                for j in range(4):
                    pt = pt2.tile([P, DH + 1], fp32, tag="pt2")
                    nc.tensor.transpose(
                        pt[:],
                        o_sb[:, P * j : P * (j + 1)],
                        id_f32[0 : DH + 1, 0 : DH + 1],
                    )
                    recip = osb.tile([P, 1], fp32, tag="recip")
                    nc.vector.reciprocal(recip[:], pt[:, DH : DH + 1])
                    nc.vector.tensor_scalar_mul(
                        out_sb[:, 4 * c + j, :], pt[:, 0:DH], recip[:]
                    )
            nc.sync.dma_start(
                out_d[:].rearrange("(t p) d -> p t d", p=P), out_sb[:]
            )

    nc.compile()
    return nc


_NC_CACHE = None


def kernel(**inputs) -> np.ndarray:
    global _NC_CACHE
    from concourse.bass_utils import run_bass_kernel_spmd

    if _NC_CACHE is None:
        _NC_CACHE = _build()
    nc = _NC_CACHE

    in_maps = []
    for b in range(B):
        m = {
            "q": np.ascontiguousarray(inputs["q"][b], dtype=np.float32),
            "k": np.ascontiguousarray(inputs["k"][b], dtype=np.float32),
            "v": np.ascontiguousarray(inputs["v"][b], dtype=np.float32),
        }
        for w in ("Wq", "Wk", "Wv", "bq", "bk", "bv"):
            m[w] = np.ascontiguousarray(inputs[w], dtype=np.float32)
        in_maps.append(m)

    res = run_bass_kernel_spmd(nc, in_maps, core_ids=list(range(B)))
    out = np.stack([res.results[b]["out"] for b in range(B)], axis=0)
    return out
